# revision 53
# baseline (speedup 1.0000x reference)
"""Trainium2 Bass kernel for nn_Basic_Block_v1 (spatial/spectral Mamba2 block).

Sharding: data-parallel over batch (16 samples) across 8 NeuronCores,
2 samples per core; all parameters replicated. Heavy math in bf16 on the
TensorEngine (1 cyc/row vs 4 for fp32); the SSD decay cumsum path stays fp32.
Depthwise convs are folded into the in_proj matmuls via host-side weight
scaling with shifted moving operands (zero-padded token axis).
"""
import sys
sys.path.insert(0, '/opt/trn_rl_repo')
import json

import numpy as np
import ml_dtypes

import concourse.bass as bass
import concourse.mybir as mybir
from concourse import tile
from concourse.bass_utils import run_bass_kernel_spmd

F32 = mybir.dt.float32
BF16 = mybir.dt.bfloat16
I32 = mybir.dt.int32
AF = mybir.ActivationFunctionType
ALU = mybir.AluOpType
AX = mybir.AxisListType
NPBF = ml_dtypes.bfloat16

NCORES = 8
BPC = 2          # batch per core
L = 256          # spatial tokens
C = 128          # channels
H1 = 4           # spa heads
H2 = 8           # spe heads
EPS = 1e-5
NEG = -88.0

# ---------------------------------------------------------------------------
# walrus in this container supports only ONE sync-wait per instruction;
# split extra waits emitted by the Tile scheduler onto preceding NoOps.
_WAIT_LIMIT = 1
_orig_to_json = bass.Bass.to_json_bytes


def _fix_block(b, ctr):
    insts = b.get('instructions')
    if insts:
        out = []
        for ins in insts:
            si = ins.get('sync_info')
            waits = (si or {}).get('on_wait') or []
            if len(waits) > _WAIT_LIMIT:
                while len(waits) > _WAIT_LIMIT:
                    chunk, waits = waits[:_WAIT_LIMIT], waits[_WAIT_LIMIT:]
                    ctr[0] += 1
                    out.append({
                        "debug": ins.get("debug"),
                        "engine": ins["engine"],
                        "ins": [],
                        "name": f"I-wsplit{ctr[0]}",
                        "opcode": "NoOp",
                        "outs": [],
                        "text_hint": "wsplit",
                        "sync_info": {"on_update": [], "on_wait": chunk},
                    })
                si['on_wait'] = waits
            out.append(ins)
        b['instructions'] = out
    for sb in b.get('blocks') or []:
        _fix_block(sb, ctr)


def _patched_to_json(self, *a, **k):
    raw = _orig_to_json(self, *a, **k)
    d = json.loads(raw)
    ctr = [0]
    for f in d.get('functions', []):
        for b in f.get('blocks', []):
            _fix_block(b, ctr)
    if ctr[0] == 0:
        return raw
    return json.dumps(d).encode()


bass.Bass.to_json_bytes = _patched_to_json


# ---------------------------------------------------------------------------
def _sincos_2d(dim, Hg):
    def e1(d, pos):
        omega = 1.0 / (10000.0 ** (np.arange(d // 2, dtype=np.float64) / (d / 2.0)))
        out = pos[:, None] * omega[None, :]
        return np.concatenate([np.sin(out), np.cos(out)], axis=-1)
    gh, gw = np.meshgrid(np.arange(Hg), np.arange(Hg), indexing='ij')
    emb = np.concatenate([e1(dim // 2, gh.reshape(-1)), e1(dim // 2, gw.reshape(-1))], axis=-1)
    return emb.astype(np.float32)


def host_constants():
    d = {}
    d['pe_fm'] = np.ascontiguousarray(_sincos_2d(C, 16).T).astype(NPBF)   # [128, 256]
    d['ident'] = np.eye(128, dtype=np.float32).astype(NPBF)
    d['identF'] = np.eye(16, dtype=np.float32)
    iota = np.arange(L, dtype=np.float32)
    d['iotaC'] = np.stack([iota[:128], iota[128:]], axis=1).copy()        # [128, 2] f32
    # Minf[sp][st][t] = +30 (pass) if (st*128+sp) <= t else NEG (mask)  (spa)
    sidx = np.arange(L)[:, None]
    tidx = np.arange(L)[None, :]
    m = np.where(sidx <= tidx, 30.0, NEG).astype(np.float32)              # [s, t]
    d['minf_spa'] = np.stack([m[:128], m[128:]], axis=1).astype(np.float16)
    s2 = np.arange(C)[:, None]
    t2 = np.arange(C)[None, :]
    d['minf_spe'] = np.where(s2 <= t2, 30.0, NEG).astype(np.float16)      # [128, 128]
    EA = np.zeros((8, 128), np.float32)
    for h in range(8):
        EA[h, h * 16:(h + 1) * 16] = 1.0
    d['E_attn'] = EA.astype(NPBF)                                         # [8, 128]
    d['Emask_q'] = EA.T.copy().astype(NPBF)                               # [128, 8]
    return d


COL_ORDER = (
    [f"spa_dtb{i}" for i in range(2)] + [f"spa_alog{i}" for i in range(2)]
    + [f"spa_cb{i}_{b}" for i in range(2) for b in range(2)]
    + [f"spa_cbB{i}" for i in range(2)] + [f"spa_cbC{i}" for i in range(2)]
    + [f"spa_dpc{i}_{j}" for i in range(2) for j in range(2)]
    + [f"spa_rwc{i}_{j}" for i in range(2) for j in range(2)]
    + [f"spe_dtb{i}" for i in range(2)] + [f"spe_alog{i}" for i in range(2)]
    + [f"spe_cb{i}_{b}" for i in range(2) for b in range(4)]
    + [f"spe_cbB{i}" for i in range(2)] + [f"spe_cbC{i}" for i in range(2)]
    + [f"spe_dpc{i}_{j}" for i in range(2) for j in range(4)]
    + [f"spe_rwc{i}_{j}" for i in range(2) for j in range(4)]
    + ["lnw_spa0", "lnw_spa1", "lnw_norm",
       "cprj_b", "aq_b", "ak_b", "av_b", "ao_b",
       "sq_b0", "sq_b1", "sk_b0", "sk_b1"]
)
CIDX = {k: ix for ix, k in enumerate(COL_ORDER)}


def prep_weights(inp):
    """Host-side layout prep: bf16 casts, transposes, conv folding."""
    w = {}
    bf = lambda a: np.ascontiguousarray(a).astype(NPBF)
    # ---- spa mamba: in_w [2, 644, 128]; conv folded into xBC blocks ----
    spa_z = np.zeros((2, 128, 256), np.float32)
    spa_xc = np.zeros((2, 4, 128, 384), np.float32)
    spa_dt = np.zeros((2, 128, 4), np.float32)
    for i in range(2):
        W = np.asarray(inp['spa_in_w'][i], np.float32)          # [644, 128]
        cw = np.asarray(inp['spa_conv_w'][i], np.float32)       # [384, 4]
        spa_z[i] = W[0:256].T
        for k in range(4):
            spa_xc[i, k] = (W[256:640] * cw[:, k:k + 1]).T
        spa_dt[i] = W[640:644].T
    w['spa_z_w'] = bf(spa_z.transpose(1, 0, 2))          # [128, 2, 256]
    w['spa_xc_w'] = bf(spa_xc.transpose(2, 0, 1, 3))     # [128, 2, 4, 384]
    w['spa_dt_w'] = bf(spa_dt.transpose(1, 0, 2))        # [128, 2, 4]
    sow = np.transpose(inp['spa_out_w'], (0, 2, 1)).reshape(2, 2, 128, 128)
    w['spa_out_pk'] = bf(np.transpose(sow, (2, 0, 1, 3)))       # [128, 2, 2, 128]
    # ---- spe mamba: in_w [2, 1160, 256] ----
    spe_z = np.zeros((2, 2, 128, 512), np.float32)
    spe_xc = np.zeros((2, 4, 2, 128, 640), np.float32)
    spe_dt = np.zeros((2, 2, 128, 8), np.float32)
    for i in range(2):
        W = np.asarray(inp['spe_in_w'][i], np.float32)          # [1160, 256]
        cw = np.asarray(inp['spe_conv_w'][i], np.float32)       # [640, 4]
        for kc in range(2):
            cs = slice(kc * 128, (kc + 1) * 128)
            spe_z[i, kc] = W[0:512, cs].T
            spe_dt[i, kc] = W[1152:1160, cs].T
            for k in range(4):
                spe_xc[i, k, kc] = (W[512:1152, cs] * cw[:, k:k + 1]).T
    w['spe_z_w'] = bf(spe_z.transpose(2, 0, 1, 3))       # [128, 2, 2, 512]
    w['spe_xc_w'] = bf(spe_xc.transpose(3, 0, 1, 2, 4))  # [128, 2, 4, 2, 640]
    w['spe_dt_w'] = bf(spe_dt.transpose(2, 0, 1, 3))     # [128, 2, 2, 8]
    sew = np.transpose(inp['spe_out_w'], (0, 2, 1)).reshape(2, 4, 128, 256)
    w['spe_out_pk'] = bf(np.transpose(sew, (2, 0, 1, 3)))       # [128, 2, 4, 256]
    w['spe_ln_wB'] = bf(np.broadcast_to(
        inp['spe_ln_w'][:, None, :], (2, 128, 256)).transpose(1, 0, 2))
    w['spe_ln_bB'] = bf(np.broadcast_to(
        inp['spe_ln_b'][:, None, :], (2, 128, 256)).transpose(1, 0, 2))
    # ---- attention / head ----
    w['cprj_pk'] = bf(np.transpose(inp['cprj_w'], (2, 1, 0)).transpose(1, 0, 2))
    for nm in ('aq', 'ak', 'av', 'ao'):
        w[nm + 'T'] = bf(inp[nm + '_w'].T)
    for nm in ('sq', 'sk', 'sv', 'so'):
        wt_ = np.asarray(inp[nm + '_w'], np.float32).T.reshape(2, 128, 256)
        w[nm + 'T'] = bf(wt_.transpose(1, 0, 2))
    w['svbB'] = bf(np.broadcast_to(inp['sv_b'][None, :], (128, 256)))
    w['sobB'] = bf(np.broadcast_to(inp['so_b'][None, :], (128, 256)))
    w['dsw_pk'] = bf(np.asarray(inp['ds_conv_w'], np.float32)
                     .reshape(9, 128, 128).transpose(1, 0, 2))
    w['ds_ln_wB'] = np.ascontiguousarray(
        np.broadcast_to(inp['ds_ln_w'][None, :], (64, 128))).astype(np.float32)
    w['ds_ln_bB'] = np.ascontiguousarray(
        np.broadcast_to(inp['ds_ln_b'][None, :], (64, 128))).astype(np.float32)
    # partition-dim layernorm stationary: row0 = -w (sign trick), row1 = b
    lnwb = np.zeros((2, 3, 128), np.float32)
    lnwb[0, 0], lnwb[1, 0] = -np.asarray(inp['spa_ln_w'][0]), inp['spa_ln_b'][0]
    lnwb[0, 1], lnwb[1, 1] = -np.asarray(inp['spa_ln_w'][1]), inp['spa_ln_b'][1]
    lnwb[0, 2], lnwb[1, 2] = -np.asarray(inp['norm_w']), inp['norm_b']
    w['lnwb'] = bf(lnwb)
    # ---- f32 scalar column pack ----
    cols = {}
    for i in range(2):
        cols[f"spa_dtb{i}"] = inp['spa_dt_bias'][i]
        cols[f"spa_alog{i}"] = inp['spa_A_log'][i]
        cb = np.asarray(inp['spa_conv_b'][i], np.float32)
        cols[f"spa_cb{i}_0"] = cb[0:128]
        cols[f"spa_cb{i}_1"] = cb[128:256]
        cols[f"spa_cbB{i}"] = cb[256:320]
        cols[f"spa_cbC{i}"] = cb[320:384]
        for j in range(2):
            cols[f"spa_dpc{i}_{j}"] = np.repeat(inp['spa_D'][i], 64)[j * 128:(j + 1) * 128]
            cols[f"spa_rwc{i}_{j}"] = inp['spa_rms_w'][i, j * 128:(j + 1) * 128]
        cols[f"spe_dtb{i}"] = inp['spe_dt_bias'][i]
        cols[f"spe_alog{i}"] = inp['spe_A_log'][i]
        cb2 = np.asarray(inp['spe_conv_b'][i], np.float32)
        for b in range(4):
            cols[f"spe_cb{i}_{b}"] = cb2[b * 128:(b + 1) * 128]
        cols[f"spe_cbB{i}"] = cb2[512:576]
        cols[f"spe_cbC{i}"] = cb2[576:640]
        for j in range(4):
            cols[f"spe_dpc{i}_{j}"] = np.repeat(inp['spe_D'][i], 64)[j * 128:(j + 1) * 128]
            cols[f"spe_rwc{i}_{j}"] = inp['spe_rms_w'][i, j * 128:(j + 1) * 128]
    cols["lnw_spa0"] = inp['spa_ln_w'][0]
    cols["lnw_spa1"] = inp['spa_ln_w'][1]
    cols["lnw_norm"] = inp['norm_w']
    cols["cprj_b"] = inp['cprj_b']
    for nm in ('aq', 'ak', 'av', 'ao'):
        cols[nm + "_b"] = inp[nm + '_b']
    cols["sq_b0"] = inp['sq_b'][0:128]
    cols["sq_b1"] = inp['sq_b'][128:256]
    cols["sk_b0"] = inp['sk_b'][0:128]
    cols["sk_b1"] = inp['sk_b'][128:256]
    pk = np.zeros((128, len(COL_ORDER)), np.float32)
    for k, v in cols.items():
        v = np.asarray(v, np.float32)
        pk[0:v.shape[0], CIDX[k]] = v
    w['colpak'] = pk
    return w


WSHAPES = {
    'spa_z_w': ([128, 2, 256], BF16), 'spa_xc_w': ([128, 2, 4, 384], BF16),
    'spa_dt_w': ([128, 2, 4], BF16), 'spa_out_pk': ([128, 2, 2, 128], BF16),
    'spe_z_w': ([128, 2, 2, 512], BF16), 'spe_xc_w': ([128, 2, 4, 2, 640], BF16),
    'spe_dt_w': ([128, 2, 2, 8], BF16), 'spe_out_pk': ([128, 2, 4, 256], BF16),
    'spe_ln_wB': ([128, 2, 256], BF16), 'spe_ln_bB': ([128, 2, 256], BF16),
    'cprj_pk': ([128, 5, 128], BF16),
    'aqT': ([128, 128], BF16), 'akT': ([128, 128], BF16),
    'avT': ([128, 128], BF16), 'aoT': ([128, 128], BF16),
    'sqT': ([128, 2, 256], BF16), 'skT': ([128, 2, 256], BF16),
    'svT': ([128, 2, 256], BF16), 'soT': ([128, 2, 256], BF16),
    'svbB': ([128, 256], BF16), 'sobB': ([128, 256], BF16),
    'dsw_pk': ([128, 9, 128], BF16),
    'ds_ln_wB': ([64, 128], F32), 'ds_ln_bB': ([64, 128], F32),
    'lnwb': ([2, 3, 128], BF16), 'colpak': ([128, len(COL_ORDER)], F32),
}
CSHAPES = {
    'pe_fm': ([128, 256], BF16), 'ident': ([128, 128], BF16),
    'identF': ([16, 16], F32), 'iotaC': ([128, 2], F32),
    'minf_spa': ([128, 2, 256], mybir.dt.float16),
    'minf_spe': ([128, 128], mybir.dt.float16),
    'E_attn': ([8, 128], BF16), 'Emask_q': ([128, 8], BF16),
}

# ---------------------------------------------------------------------------
def build_program(taps=()):
    """Builds the per-core SPMD Bass program."""
    nc = bass.Bass()

    def din(name, shape, dt=F32):
        return nc.dram_tensor(name, shape, dt, kind="ExternalInput")

    x2 = din("x2", [BPC, C, L], BF16)
    idx = din("idx", [BPC, L], I32)
    inv = din("inv", [BPC, L], I32)
    cst_t = {k: din(k, shp, dt) for k, (shp, dt) in CSHAPES.items()}
    w_t = {k: din(k, shp, dt) for k, (shp, dt) in WSHAPES.items()}
    out = nc.dram_tensor("out", [BPC, 8, 8, C], F32, kind="ExternalOutput")
    tap_t = {}

    with tile.TileContext(nc) as tc:
        import contextlib
        stk = contextlib.ExitStack()
        sb = stk.enter_context(tc.tile_pool(name="sb", bufs=2))
        wb = stk.enter_context(tc.tile_pool(name="wb", bufs=1))
        psA = stk.enter_context(tc.tile_pool(name="psA", bufs=2, space="PSUM"))
        psB = stk.enter_context(tc.tile_pool(name="psB", bufs=3, space="PSUM"))
        psS = stk.enter_context(tc.tile_pool(name="psS", bufs=2, space="PSUM"))
        psD = stk.enter_context(tc.tile_pool(name="psD", bufs=1, space="PSUM"))

        def T(shape, tag, dt=BF16, bufs=None):
            return sb.tile(shape, dt, tag=tag, name=tag, bufs=bufs)

        def W(shape, tag, dt=BF16):
            return wb.tile(shape, dt, tag=tag, name=tag, bufs=1)

        def PA(shape=(128, 512), dt=F32):
            return psA.tile(list(shape), dt, tag="A", name="pa",
                            padded_shape=[128, 512 if dt == F32 else 1024])

        def PB(shape=(128, 512), dt=F32):
            return psB.tile(list(shape), dt, tag="B", name="pb",
                            padded_shape=[128, 512 if dt == F32 else 1024])

        def PS(shape=(16, 512), dt=F32):
            return psS.tile(list(shape), dt, tag="S", name="ps",
                            padded_shape=[shape[0], 512])

        dma = nc.sync.dma_start
        V = nc.vector
        S = nc.scalar
        G = nc.gpsimd
        MM = nc.tensor.matmul
        TR = nc.tensor.transpose

        # ---------- load constants + weights (single DMA per tensor) ----------
        ct = {}
        for k, (shp, dt) in CSHAPES.items():
            ct[k] = W(shp, "c_" + k, dt)
            dma(ct[k][:], cst_t[k][:])
        wt = {}
        for k, (shp, dt) in WSHAPES.items():
            wt[k] = W(shp, "w_" + k, dt)
            dma(wt[k][:], w_t[k][:])
        colpak = wt['colpak']

        def col(key, p=128):
            return colpak[0:p, CIDX[key]:CIDX[key] + 1]

        onesB = W([128, 128], "onesB", BF16)
        V.memset(onesB[:], 1.0)
        onescolB = onesB[:, 0:1]
        onesrowB = onesB[0:1, :]
        onesF = W([1, 128], "onesF", F32)
        V.memset(onesF[:], 1.0)
        onecolF = W([128, 1], "onecolF", F32)
        V.memset(onecolF[:], 1.0)
        epscol = W([128, 1], "epscol", F32)
        V.memset(epscol[:], EPS)
        ident = ct['ident']
        identF = ct['identF']

        # layer-constant A-exponentials
        eA_spa = W([4, 2], "eA_spa", F32)
        eA_spe = W([8, 2], "eA_spe", F32)
        for i in range(2):
            S.activation(eA_spa[:, i:i + 1], col(f"spa_alog{i}", 4), AF.Exp)
            S.activation(eA_spe[:, i:i + 1], col(f"spe_alog{i}", 8), AF.Exp)

        def tap(name, src_ap, shape):
            # stage through f32 + DMA out (debug only)
            if name not in taps:
                return
            st_ = T(list(shape), "tapstage", F32)
            S.copy(st_[:], src_ap)
            t = nc.dram_tensor("t_" + name, list(shape), F32, kind="ExternalOutput")
            tap_t[name] = t
            dma(t[:], st_[:])

        # ---------- stage 0: embed + permute ----------
        xb = T([128, BPC, L], "xb")
        for s in range(BPC):
            dma(xb[:, s, :], x2[s])
        x0 = T([128, BPC, L], "x0")
        V.tensor_tensor(
            x0[:], xb[:],
            ct['pe_fm'][:].unsqueeze(1).to_broadcast((128, BPC, L)),
            op=ALU.add)

        idxr = T([1, BPC, L], "irow_raw", I32, bufs=1)
        dma(idxr[:], idx[None, :, :])
        idxf = T([1, BPC, L], "irow_f", F32, bufs=1)
        V.tensor_copy(idxf[:], idxr[:])

        xs = T([128, BPC, L], "xs")
        for s in range(BPC):
            idxB = PB()
            MM(idxB[:, 0:L], onesF[:], idxf[:, s, :], start=True, stop=True)
            PmT = T([128, 2, L], "perm_oh")
            for st in range(2):
                V.tensor_scalar(PmT[:, st, :], idxB[:, 0:L], ct['iotaC'][:, st:st + 1],
                                None, op0=ALU.is_equal)
            x0tm = T([128, 2, 128], "tm_tmp")
            for tt in range(2):
                ptr = PB((128, 128), BF16)
                TR(ptr[:, 0:128], x0[:, s, tt * 128:(tt + 1) * 128], ident[:])
                S.copy(x0tm[:, tt, :], ptr[:, 0:128])
            pxs = PB((128, 256))
            for st in range(2):
                MM(pxs[:], x0tm[:, st, :], PmT[:, st, :],
                   start=(st == 0), stop=(st == 1))
            S.copy(xs[:, s, :], pxs[:])
        tap("xs0", xs[:].rearrange("p s t -> p (s t)"), (128, 512))

        # ================= partition-dim layernorm =================
        lnrhs = T([2, 512], "ln_rhs", BF16, bufs=2)

        def part_ln(xsrc, lnidx, dst):
            """LN over channel (partition) dim. xsrc/dst: [128, 2, 256] views."""
            xflat = xsrc.rearrange("p s t -> p (s t)") if len(xsrc.shape) == 3 else xsrc
            sq = T([128, 512], "ln_sq")
            S.activation(sq[:], xflat, AF.Square)
            msum = PS((1, 512))
            MM(msum[:], onescolB, xflat, start=True, stop=True)
            ssum = PS((1, 512))
            MM(ssum[:], onescolB, sq[:], start=True, stop=True)
            murow = T([1, 512], "ln_mu", F32, bufs=1)
            S.activation(murow[:], msum[:], AF.Copy, scale=1.0 / 128)
            mu2 = T([1, 512], "ln_mu2", F32, bufs=1)
            S.activation(mu2[:], murow[:], AF.Square)
            var = T([1, 512], "ln_var", F32, bufs=1)
            V.scalar_tensor_tensor(var[:], ssum[:], 1.0 / 128, mu2[:],
                                   op0=ALU.mult, op1=ALU.subtract)
            lnv = T([1, 512], "ln_lnv", F32, bufs=1)
            S.activation(lnv[:], var[:], AF.Ln, bias=epscol[0:1, 0:1])
            rstd = T([1, 512], "ln_rstd", BF16)
            S.activation(rstd[:], lnv[:], AF.Exp, scale=-0.5)
            V.memset(lnrhs[:], 1.0)
            V.tensor_tensor(lnrhs[0:1, :], murow[:], rstd[:], op=ALU.mult)
            Rp = PA()
            MM(Rp[:], wt['lnwb'][:, lnidx, :], lnrhs[:], start=True, stop=True)
            rstdB = PA()
            MM(rstdB[:], onesrowB, rstd[:], start=True, stop=True)
            wcol = col(("lnw_spa0", "lnw_spa1", "lnw_norm")[lnidx])
            tmp = T([128, 512], "ln_tmp")
            V.tensor_tensor(tmp[:], xflat, rstdB[:], op=ALU.mult)
            if len(dst.shape) == 3:
                V.scalar_tensor_tensor(
                    dst, tmp[:].rearrange("p (s t) -> p s t", s=2), wcol,
                    Rp[:].rearrange("p (s t) -> p s t", s=2),
                    op0=ALU.mult, op1=ALU.add)
            else:
                V.scalar_tensor_tensor(dst, tmp[:], wcol, Rp[:],
                                       op0=ALU.mult, op1=ALU.add)

        # ================= spa mamba =================
        def spa_mamba(i, xs):
            xlnp = T([128, BPC, 259], "xlnp")
            V.memset(xlnp[:, :, 0:3], 0.0)
            part_ln(xs[:], i, xlnp[:, :, 3:259])
            xln = xlnp[:, :, 3:259]
            # dt path (fp32)
            pdt = PS((4, 512))
            MM(pdt[:], wt['spa_dt_w'][:, i, :], xln, start=True, stop=True)
            e1 = T([4, 512], "mb_e1", F32, bufs=1)
            S.activation(e1[:], pdt[:], AF.Exp, bias=col(f"spa_dtb{i}", 4))
            dtv = T([4, 512], "mb_dtv", F32, bufs=1)
            S.activation(dtv[:], e1[:], AF.Ln, bias=onecolF[0:4, 0:1])
            ldt = T([4, 512], "mb_ldt", F32, bufs=1)
            S.activation(ldt[:], dtv[:], AF.Ln)
            dtA = T([4, 512], "mb_dtA", F32, bufs=1)
            V.tensor_scalar(dtA[:], dtv[:], eA_spa[:, i:i + 1], -1.0,
                            op0=ALU.mult, op1=ALU.mult)
            acum = T([4, 512], "mb_acum", F32, bufs=1)
            for s in range(BPC):
                V.tensor_tensor_scan(acum[:, s * 256:(s + 1) * 256],
                                     dtA[:, s * 256:(s + 1) * 256],
                                     dtA[:, s * 256:(s + 1) * 256], 0.0,
                                     op0=ALU.add, op1=ALU.bypass)
            alt = T([4, 512], "mb_alt", F32, bufs=1)
            V.tensor_tensor(alt[:], acum[:], ldt[:], op=ALU.subtract)
            aflat = T([1, 2, 1024], "aflat", F32, bufs=1)
            for s in range(BPC):
                dma(aflat[0:1, s, :].rearrange("o (p f) -> o p f", p=4),
                    acum[:, s * 256:(s + 1) * 256])
            zsil = T([128, 2, 512], "mb_zsil")
            for j in range(2):
                pz = PA()
                MM(pz[:], wt['spa_z_w'][:, i, j * 128:(j + 1) * 128], xln,
                   start=True, stop=True)
                S.activation(zsil[:, j, :], pz[:], AF.Silu)
            # xBC blocks with conv folded: x halves + B + C
            xcx = T([128, 2, 2, 256], "mb_xcx")     # [p, j, s, t]
            xcB = T([64, 2, 256], "mb_xcB")
            xcC = T([64, 2, 256], "mb_xcC")
            blocks = [(0, 128, xcx[:, 0, :, :], col(f"spa_cb{i}_0")),
                      (128, 128, xcx[:, 1, :, :], col(f"spa_cb{i}_1")),
                      (256, 64, xcB[:], col(f"spa_cbB{i}", 64)),
                      (320, 64, xcC[:], col(f"spa_cbC{i}", 64))]
            for c0, rows, dst, cb in blocks:
                px = PA((rows, 512))
                for k in range(4):
                    MM(px[:].rearrange("p (s t) -> p s t", s=2),
                       wt['spa_xc_w'][:, i, k, c0:c0 + rows],
                       xlnp[:, :, k:k + 256],
                       start=(k == 0), stop=(k == 3))
                S.activation(dst.rearrange("p s t -> p (s t)"), px[:],
                             AF.Silu, bias=cb)
            ynt = T([128, 2, 2, 256], "mb_ynt")     # [p, j, s, t]
            for s in range(BPC):
                # acum+dt transposed: [tok, st, 8] f32 (cols 0:4 acum, 4:8 dt)
                altT = T([128, 2, 4], "spa_altT", F32)
                for st in range(2):
                    csl = slice(s * 256 + st * 128, s * 256 + (st + 1) * 128)
                    ptr = PB((128, 4))
                    TR(ptr[:, 0:4], alt[:, csl], identF[0:4, 0:4])
                    S.copy(altT[:, st, :], ptr[:, 0:4])
                pb1 = PB()
                MM(pb1[:], onesF[:], aflat[:, s, 0:512], start=True, stop=True)
                pb2 = PB()
                MM(pb2[:], onesF[:], aflat[:, s, 512:1024], start=True, stop=True)
                # Dt = min(acum(t) - acum(p) + ln dt_p, mask); exp gives dt*decay
                Dt = T([128, 2, 4, 256], "ssd_Dt", mybir.dt.float16)
                for st in range(2):
                    for h in range(H1):
                        pbx = pb1 if h < 2 else pb2
                        V.scalar_tensor_tensor(
                            Dt[:, st, h, :], pbx[:, (h % 2) * 256:(h % 2 + 1) * 256],
                            altT[:, st, h:h + 1], ct['minf_spa'][:, st, :],
                            op0=ALU.subtract, op1=ALU.min)
                Et = T([128, 2, 4, 256], "ssd_Et")
                S.activation(Et[:].rearrange("p a h t -> p (a h t)"),
                             Dt[:].rearrange("p a h t -> p (a h t)"), AF.Exp)
                pm0s = T([128, 2, 256], "pm0s")
                for st in range(2):
                    pm0 = PB((128, 256))
                    MM(pm0[:], xcB[:, s, st * 128:(st + 1) * 128],
                       xcC[:, s, :], start=True, stop=True)
                    S.copy(pm0s[:, st, :], pm0[:])
                MT = T([128, 2, 4, 256], "ssd_MT")
                for st in range(2):
                    for h in range(H1):
                        V.tensor_tensor(MT[:, st, h, :], Et[:, st, h, :],
                                        pm0s[:, st, :], op=ALU.mult)
                # token-major xc
                xtm = T([128, 2, 2, 128], "spa_xtm")   # [tok, st, j, 128]
                for st in range(2):
                    for j in range(2):
                        ptr = PB((128, 128), BF16)
                        TR(ptr[:, 0:128],
                           xcx[:, j, s, st * 128:(st + 1) * 128], ident[:])
                        S.copy(xtm[:, st, j, :], ptr[:, 0:128])
                ypY = PA()
                for st in range(2):
                    for h in range(H1):
                        MM(ypY[(h % 2) * 64:(h % 2) * 64 + 64,
                               (h // 2) * 256:(h // 2 + 1) * 256],
                           xtm[:, st, h // 2, (h % 2) * 64:(h % 2) * 64 + 64],
                           MT[:, st, h, :],
                           start=(st == 0), stop=(st == 1),
                           tile_position=(0, (h % 2) * 64),
                           skip_group_check=True)
                ygt = T([128, 2, 256], "spa_ygt")
                for j in range(2):
                    y0 = T([128, 256], "spa_y0")
                    V.scalar_tensor_tensor(y0[:], xcx[:, j, s, :],
                                           col(f"spa_dpc{i}_{j}"),
                                           ypY[:, j * 256:(j + 1) * 256],
                                           op0=ALU.mult, op1=ALU.add)
                    V.tensor_tensor(ygt[:, j, :], y0[:],
                                    zsil[:, j, s * 256:(s + 1) * 256], op=ALU.mult)
                # gated RMS over d_inner (256)
                sqy = T([128, 2, 256], "mb_sqy")
                S.activation(sqy[:].rearrange("p j t -> p (j t)"),
                             ygt[:].rearrange("p j t -> p (j t)"), AF.Square)
                ssy = PS((1, 256))
                for j in range(2):
                    MM(ssy[:], onescolB, sqy[:, j, :],
                       start=(j == 0), stop=(j == 1))
                varr = T([1, 256], "rms_var", F32, bufs=1)
                V.tensor_scalar(varr[:], ssy[:], 1.0 / 256, EPS,
                                op0=ALU.mult, op1=ALU.add)
                rl = T([1, 256], "rms_rl", F32, bufs=1)
                S.activation(rl[:], varr[:], AF.Ln)
                rrow = T([1, 256], "rms_rrow", BF16)
                S.activation(rrow[:], rl[:], AF.Exp, scale=-0.5)
                rB = PB((128, 256))
                MM(rB[:], onesrowB, rrow[:], start=True, stop=True)
                for j in range(2):
                    V.scalar_tensor_tensor(ynt[:, j, s, :], ygt[:, j, :],
                                           col(f"spa_rwc{i}_{j}"),
                                           rB[:], op0=ALU.mult, op1=ALU.mult)
            pop = PA()
            for j in range(2):
                MM(pop[:].rearrange("p (s t) -> p s t", s=2),
                   wt['spa_out_pk'][:, i, j, :], ynt[:, j, :, :],
                   start=(j == 0), stop=(j == 1))
            h1 = T([128, 2, 256], "h1")
            V.tensor_tensor(h1[:].rearrange("p s t -> p (s t)"), pop[:],
                            xs[:].rearrange("p s t -> p (s t)"), op=ALU.add)
            return h1

        # ================= spe mamba =================
        def spe_mamba(i, h1):
            # LayerNorm over the 256 features (free dim), per sample
            mus = T([128, 2], "spe_mus", F32)
            V.tensor_reduce(mus[:], h1[:], axis=AX.X, op=ALU.add)
            sq2 = T([128, 512], "ln_sq")
            S.activation(sq2[:], h1[:].rearrange("p s t -> p (s t)"), AF.Square)
            ss2 = T([128, 2], "spe_ss2", F32)
            V.tensor_reduce(ss2[:], sq2[:].rearrange("p (s t) -> p s t", s=2),
                            axis=AX.X, op=ALU.add)
            mean = T([128, 2], "spe_mean", F32)
            V.tensor_scalar(mean[:], mus[:], 1.0 / 256, None, op0=ALU.mult)
            m2 = T([128, 2], "spe_m2", F32)
            V.tensor_tensor(m2[:], mean[:], mean[:], op=ALU.mult)
            var2 = T([128, 2], "spe_var", F32)
            V.scalar_tensor_tensor(var2[:], ss2[:], 1.0 / 256, m2[:],
                                   op0=ALU.mult, op1=ALU.subtract)
            l2t = T([128, 2], "spe_l2", F32)
            S.activation(l2t[:], var2[:], AF.Ln, bias=epscol[:, 0:1])
            rstd2 = T([128, 2], "spe_rstd", F32)
            S.activation(rstd2[:], l2t[:], AF.Exp, scale=-0.5)
            X2fp = T([128, 2, 2, 131], "x2fp")      # [feat, s, kc, 3+tok]
            V.memset(X2fp[:, :, :, 0:3], 0.0)
            for s in range(BPC):
                xn = T([128, 256], "spe_xn")
                V.tensor_scalar(xn[:], h1[:, s, :], mean[:, s:s + 1], rstd2[:, s:s + 1],
                                op0=ALU.subtract, op1=ALU.mult)
                u = T([128, 256], "spe_u")
                V.tensor_tensor(u[:], xn[:], wt['spe_ln_wB'][:, i, :], op=ALU.mult)
                xsn = T([128, 256], "spe_xsn")
                V.tensor_tensor(xsn[:], u[:], wt['spe_ln_bB'][:, i, :], op=ALU.add)
                for ft in range(2):
                    ptr = PB((128, 128), BF16)
                    TR(ptr[:, 0:128], xsn[:, ft * 128:(ft + 1) * 128], ident[:])
                    S.copy(X2fp[:, s, ft, 3:131], ptr[:, 0:128])
            # dt path (fp32)
            pdt = PS((8, 256))
            for kc in range(2):
                MM(pdt[:], wt['spe_dt_w'][:, i, kc, :], X2fp[:, :, kc, 3:131],
                   start=(kc == 0), stop=(kc == 1))
            e1 = T([8, 256], "mb_e1", F32, bufs=1)
            S.activation(e1[:], pdt[:], AF.Exp, bias=col(f"spe_dtb{i}", 8))
            dtv = T([8, 256], "mb_dtv", F32, bufs=1)
            S.activation(dtv[:], e1[:], AF.Ln, bias=onecolF[0:8, 0:1])
            ldt = T([8, 256], "mb_ldt", F32, bufs=1)
            S.activation(ldt[:], dtv[:], AF.Ln)
            dtA = T([8, 256], "mb_dtA", F32, bufs=1)
            V.tensor_scalar(dtA[:], dtv[:], eA_spe[:, i:i + 1], -1.0,
                            op0=ALU.mult, op1=ALU.mult)
            acum = T([8, 256], "mb_acum", F32, bufs=1)
            for s in range(BPC):
                V.tensor_tensor_scan(acum[:, s * 128:(s + 1) * 128],
                                     dtA[:, s * 128:(s + 1) * 128],
                                     dtA[:, s * 128:(s + 1) * 128], 0.0,
                                     op0=ALU.add, op1=ALU.bypass)
            alt = T([8, 256], "mb_alt", F32, bufs=1)
            V.tensor_tensor(alt[:], acum[:], ldt[:], op=ALU.subtract)
            aflat = T([1, 2, 1024], "aflat", F32, bufs=1)
            for s in range(BPC):
                dma(aflat[0:1, s, :].rearrange("o (p f) -> o p f", p=8),
                    acum[:, s * 128:(s + 1) * 128])
            zsil = T([128, 2, 2, 2, 128], "mb_zsil")   # [p, g, jj, s, t]
            for g in range(2):
                pz = PA()
                for jj in range(2):
                    for kc in range(2):
                        MM(pz[:].rearrange("p (a s t) -> p a s t", a=2, s=2)[:, jj, :, :],
                           wt['spe_z_w'][:, i, kc,
                                         (2 * g + jj) * 128:(2 * g + jj + 1) * 128],
                           X2fp[:, :, kc, 3:131],
                           start=(kc == 0), stop=(kc == 1), skip_group_check=True)
                S.activation(zsil[:, g, :, :, :].rearrange("p a s t -> p (a s t)"),
                             pz[:], AF.Silu)
            xc2 = T([128, 4, 2, 128], "mb_xcx")     # [p, j, s, t]
            xcB = T([64, 2, 128], "mb_xcB")
            xcC = T([64, 2, 128], "mb_xcC")
            blocks = ([(blk * 128, 128, xc2[:, blk, :, :], col(f"spe_cb{i}_{blk}"))
                       for blk in range(4)]
                      + [(512, 64, xcB[:], col(f"spe_cbB{i}", 64)),
                         (576, 64, xcC[:], col(f"spe_cbC{i}", 64))])
            for c0, rows, dst, cb in blocks:
                px = PA((rows, 256))
                for k in range(4):
                    for kc in range(2):
                        MM(px[:].rearrange("p (s t) -> p s t", s=2),
                           wt['spe_xc_w'][:, i, k, kc, c0:c0 + rows],
                           X2fp[:, :, kc, k:k + 128],
                           start=(k == 0 and kc == 0), stop=(k == 3 and kc == 1))
                S.activation(dst.rearrange("p s t -> p (s t)"), px[:],
                             AF.Silu, bias=cb)
            ynt = T([128, 4, 2, 128], "mb_ynt")     # [p, j, s, t]
            for s in range(BPC):
                altT = T([128, 8], "spe_altT", F32)
                csl = slice(s * 128, (s + 1) * 128)
                ptr = PB((128, 8))
                TR(ptr[:, 0:8], alt[:, csl], identF[0:8, 0:8])
                S.copy(altT[:], ptr[:, 0:8])
                pb1 = PB()
                MM(pb1[:], onesF[:], aflat[:, s, 0:512], start=True, stop=True)
                pb2 = PB()
                MM(pb2[:], onesF[:], aflat[:, s, 512:1024], start=True, stop=True)
                Dt = T([128, 8, 128], "ssd_Dt", mybir.dt.float16)
                for h in range(H2):
                    pbx = pb1 if h < 4 else pb2
                    V.scalar_tensor_tensor(
                        Dt[:, h, :], pbx[:, (h % 4) * 128:(h % 4 + 1) * 128],
                        altT[:, h:h + 1], ct['minf_spe'][:],
                        op0=ALU.subtract, op1=ALU.min)
                Et = T([128, 8, 128], "ssd_Et")
                S.activation(Et[:].rearrange("p h t -> p (h t)"),
                             Dt[:].rearrange("p h t -> p (h t)"), AF.Exp)
                pm0 = PB((128, 128))
                MM(pm0[:], xcB[:, s, :], xcC[:, s, :],
                   start=True, stop=True)
                pm0s = T([128, 128], "pm0s")
                S.copy(pm0s[:], pm0[:])
                MT = T([128, 8, 128], "ssd_MT")
                for h in range(H2):
                    V.tensor_tensor(MT[:, h, :], Et[:, h, :],
                                    pm0s[:], op=ALU.mult)
                xtm2 = T([128, 4, 128], "spa_xtm")  # [tok, j, 128]
                for j in range(4):
                    ptr = PB((128, 128), BF16)
                    TR(ptr[:, 0:128], xc2[:, j, s, :], ident[:])
                    S.copy(xtm2[:, j, :], ptr[:, 0:128])
                ypY = PA()
                for j in range(4):
                    for hh in range(2):
                        MM(ypY[hh * 64:hh * 64 + 64, j * 128:(j + 1) * 128],
                           xtm2[:, j, hh * 64:hh * 64 + 64],
                           MT[:, 2 * j + hh, :], start=True, stop=True,
                           tile_position=(0, hh * 64),
                           skip_group_check=True)
                ygt = T([128, 4, 128], "spe_ygt")
                for j in range(4):
                    y0 = T([128, 128], "spe_y0")
                    V.scalar_tensor_tensor(y0[:], xc2[:, j, s, :],
                                           col(f"spe_dpc{i}_{j}"),
                                           ypY[:, j * 128:(j + 1) * 128],
                                           op0=ALU.mult, op1=ALU.add)
                    V.tensor_tensor(ygt[:, j, :], y0[:],
                                    zsil[:, j // 2, j % 2, s, :], op=ALU.mult)
                sqy = T([128, 4, 128], "mb_sqy")
                S.activation(sqy[:].rearrange("p j t -> p (j t)"),
                             ygt[:].rearrange("p j t -> p (j t)"), AF.Square)
                ssy = PS((1, 128))
                for j in range(4):
                    MM(ssy[:], onescolB, sqy[:, j, :],
                       start=(j == 0), stop=(j == 3))
                varr = T([1, 128], "rms_var", F32, bufs=1)
                V.tensor_scalar(varr[:], ssy[:], 1.0 / 512, EPS,
                                op0=ALU.mult, op1=ALU.add)
                rl = T([1, 128], "rms_rl", F32, bufs=1)
                S.activation(rl[:], varr[:], AF.Ln)
                rrow = T([1, 128], "rms_rrow", BF16)
                S.activation(rrow[:], rl[:], AF.Exp, scale=-0.5)
                rB = PB((128, 128))
                MM(rB[:], onesrowB, rrow[:], start=True, stop=True)
                for j in range(4):
                    V.scalar_tensor_tensor(ynt[:, j, s, :], ygt[:, j, :],
                                           col(f"spe_rwc{i}_{j}"),
                                           rB[:], op0=ALU.mult, op1=ALU.mult)
            xs_new = T([128, 2, 256], "xs")
            for ft in range(2):
                ph2 = PB((128, 256))
                for k in range(4):
                    MM(ph2[:].rearrange("p (s t) -> p s t", s=2),
                       wt['spe_out_pk'][:, i, k, ft * 128:(ft + 1) * 128],
                       ynt[:, k, :, :], start=(k == 0), stop=(k == 3))
                h2f = T([128, 256], "spe_h2f")
                S.copy(h2f[:], ph2[:])
                for s in range(BPC):
                    ptr = PB((128, 128), BF16)
                    TR(ptr[:, 0:128], h2f[:, s * 128:(s + 1) * 128], ident[:])
                    V.tensor_tensor(xs_new[:, s, ft * 128:(ft + 1) * 128],
                                    ptr[:, 0:128], h1[:, s, ft * 128:(ft + 1) * 128],
                                    op=ALU.add)
            return xs_new

        # ================= layers =================
        cur = xs
        for i in range(2):
            h1 = spa_mamba(i, cur)
            tap(f"h1_{i}", h1[:].rearrange("p s t -> p (s t)"), (128, 512))
            cur = spe_mamba(i, h1)
            tap(f"xsl{i + 1}", cur[:].rearrange("p s t -> p (s t)"), (128, 512))

        # ================= final LN =================
        xf = T([128, 2, 256], "xf")
        part_ln(cur[:], 2, xf[:])
        xfl = xf[:].rearrange("p s t -> p (s t)")
        tap("xf", xfl, (128, 512))

        # ================= spa attention (center query) =================
        pctr = PS((128, 2))
        for l in range(5):
            MM(pctr[:], wt['cprj_pk'][:, l, :], xf[:, :, l],
               start=(l == 0), stop=(l == 4))
        ctr = T([128, 2], "at_ctr")
        S.activation(ctr[:], pctr[:], AF.Identity, bias=col("cprj_b"))
        pq = PS((128, 2))
        MM(pq[:], wt['aqT'][:], ctr[:], start=True, stop=True)
        qsb = T([128, 2], "at_q")
        S.activation(qsb[:], pq[:], AF.Identity, bias=col("aq_b"))
        pk = PA()
        MM(pk[:], wt['akT'][:], xfl, start=True, stop=True)
        Ksb = T([128, 2, 256], "at_K")
        S.activation(Ksb[:].rearrange("p s t -> p (s t)"), pk[:], AF.Identity,
                     bias=col("ak_b"))
        pv = PA()
        MM(pv[:], wt['avT'][:], xfl, start=True, stop=True)
        Vsb = T([128, 2, 256], "at_V")
        S.activation(Vsb[:].rearrange("p s t -> p (s t)"), pv[:], AF.Identity,
                     bias=col("av_b"))
        vo = T([128, 2, 256], "at_vo")
        for s in range(BPC):
            qd = T([128, 8], "at_qd")
            V.tensor_tensor(qd[:], qsb[:, s:s + 1].to_broadcast((128, 8)),
                            ct['Emask_q'][:], op=ALU.mult)
            plg = PS((8, 256))
            MM(plg[:], qd[:], Ksb[:, s, :], start=True, stop=True)
            nm = T([8, 1], "at_nm", F32)
            V.tensor_reduce(nm[:], plg[:], axis=AX.X, op=ALU.max, negate=True)
            nm4 = T([8, 1], "at_nm4", F32)
            V.tensor_scalar(nm4[:], nm[:], 0.25, None, op0=ALU.mult)
            ex = T([8, 256], "at_ex")
            S.activation(ex[:], plg[:], AF.Exp, bias=nm4[:, 0:1], scale=0.25)
            sm = T([8, 1], "at_sm", F32)
            V.tensor_reduce(sm[:], ex[:], axis=AX.X, op=ALU.add)
            rc = T([8, 1], "at_rc", F32)
            V.reciprocal(rc[:], sm[:])
            aw = T([8, 256], "at_aw")
            V.tensor_scalar(aw[:], ex[:], rc[:, 0:1], None, op0=ALU.mult)
            patB = PB((128, 256))
            MM(patB[:], ct['E_attn'][:], aw[:], start=True, stop=True)
            V.tensor_tensor(vo[:, s, :], Vsb[:, s, :], patB[:], op=ALU.mult)
        pao = PA()
        MM(pao[:], wt['aoT'][:], vo[:].rearrange("p s t -> p (s t)"),
           start=True, stop=True)
        xa = T([128, 2, 256], "xa")
        V.scalar_tensor_tensor(xa[:].rearrange("p s t -> p (s t)"), pao[:],
                               col("ao_b"), xfl, op0=ALU.add, op1=ALU.add)
        tap("xa", xa[:].rearrange("p s t -> p (s t)"), (128, 512))

        # ================= spe attention =================
        X2a = T([128, 2, 2, 128], "x2fp")
        for s in range(BPC):
            for ft in range(2):
                ptr = PB((128, 128), BF16)
                TR(ptr[:, 0:128], xa[:, s, ft * 128:(ft + 1) * 128], ident[:])
                S.copy(X2a[:, s, ft, :], ptr[:, 0:128])
        q2 = T([128, 2, 2, 128], "sp2_q2")   # [p, ot, s, t]
        k2 = T([128, 2, 2, 128], "sp2_k2")
        for ot in range(2):
            pq2 = PB((128, 256))
            for ft in range(2):
                MM(pq2[:].rearrange("p (s t) -> p s t", s=2),
                   wt['sqT'][:, ft, ot * 128:(ot + 1) * 128],
                   X2a[:, :, ft, :], start=(ft == 0), stop=(ft == 1))
            S.activation(q2[:, ot, :, :].rearrange("p s t -> p (s t)"),
                         pq2[:], AF.Identity, bias=col(f"sq_b{ot}"))
            pk2 = PB((128, 256))
            for ft in range(2):
                MM(pk2[:].rearrange("p (s t) -> p s t", s=2),
                   wt['skT'][:, ft, ot * 128:(ot + 1) * 128],
                   X2a[:, :, ft, :], start=(ft == 0), stop=(ft == 1))
            S.activation(k2[:, ot, :, :].rearrange("p s t -> p (s t)"),
                         pk2[:], AF.Identity, bias=col(f"sk_b{ot}"))
        xs2 = T([128, 2, 256], "xs2")
        for s in range(BPC):
            pv2 = PB((128, 256))
            for ft in range(2):
                MM(pv2[:], X2a[:, s, ft, :], wt['svT'][:, ft, :],
                   start=(ft == 0), stop=(ft == 1))
            v2 = T([128, 256], "sp2_v2")
            V.tensor_tensor(v2[:], pv2[:], wt['svbB'][:], op=ALU.add)
            pa2 = PB((128, 128))
            for ot in range(2):
                MM(pa2[:, 0:128], q2[:, ot, s, :], k2[:, ot, s, :],
                   start=(ot == 0), stop=(ot == 1))
            nm = T([128, 1], "sp2_nm", F32)
            V.tensor_reduce(nm[:], pa2[:, 0:128], axis=AX.X, op=ALU.max, negate=True)
            nm16 = T([128, 1], "sp2_nm16", F32)
            V.tensor_scalar(nm16[:], nm[:], 1.0 / 16, None, op0=ALU.mult)
            ex = T([128, 128], "sp2_ex")
            S.activation(ex[:], pa2[:, 0:128], AF.Exp, bias=nm16[:, 0:1], scale=1.0 / 16)
            sm = T([128, 1], "sp2_sm", F32)
            V.tensor_reduce(sm[:], ex[:], axis=AX.X, op=ALU.add)
            rc = T([128, 1], "sp2_rc", F32)
            V.reciprocal(rc[:], sm[:])
            a2 = T([128, 128], "sp2_a2")
            V.tensor_scalar(a2[:], ex[:], rc[:, 0:1], None, op0=ALU.mult)
            pa2T = PB((128, 128), BF16)
            TR(pa2T[:, 0:128], a2[:], ident[:])
            a2T = T([128, 128], "sp2_a2T")
            S.copy(a2T[:], pa2T[:, 0:128])
            o2 = T([128, 2, 128], "sp2_o2")
            for ot in range(2):
                po2 = PB((128, 128))
                MM(po2[:, 0:128], v2[:, ot * 128:(ot + 1) * 128], a2T[:],
                   start=True, stop=True)
                S.copy(o2[:, ot, :], po2[:, 0:128])
            po3 = PB((128, 256))
            for ot in range(2):
                MM(po3[:], o2[:, ot, :], wt['soT'][:, ot, :],
                   start=(ot == 0), stop=(ot == 1))
            t3 = T([128, 256], "sp2_t3")
            V.tensor_tensor(t3[:], po3[:], wt['sobB'][:], op=ALU.add)
            V.tensor_tensor(xs2[:, s, :], t3[:], xa[:, s, :], op=ALU.add)
        tap("xs2", xs2[:].rearrange("p s t -> p (s t)"), (128, 512))

        # ================= downsample =================
        invr = T([1, BPC, L], "irow_raw", I32, bufs=1)
        dma(invr[:], inv[None, :, :])
        invf = T([1, BPC, L], "irow_f", F32, bufs=1)
        V.tensor_copy(invf[:], invr[:])
        pds = psD.tile([64, 512], F32, tag="ds", name="pds")
        for s in range(BPC):
            invB = PB()
            MM(invB[:, 0:L], onesF[:], invf[:, s, :], start=True, stop=True)
            QT = T([128, 2, 256], "perm_oh")
            for tt in range(2):
                V.tensor_scalar(QT[:, tt, :], invB[:, 0:L], ct['iotaC'][:, tt:tt + 1],
                                None, op0=ALU.is_equal)
            tmv = T([128, 2, 128], "tm_tmp")
            for tt in range(2):
                ptr = PB((128, 128), BF16)
                TR(ptr[:, 0:128], xs2[:, s, tt * 128:(tt + 1) * 128], ident[:])
                S.copy(tmv[:, tt, :], ptr[:, 0:128])
            pxr = PB((128, 256))
            for tt in range(2):
                MM(pxr[:], tmv[:, tt, :], QT[:, tt, :],
                   start=(tt == 0), stop=(tt == 1))
            xrp = T([128, 324], "ds_xrp")
            V.memset(xrp[:], 0.0)
            xr3 = xrp[:].rearrange("p (h w) -> p h w", h=18)
            S.copy(xr3[:, 1:17, 1:17], pxr[:].rearrange("p (h w) -> p h w", h=16))
            for kh in range(3):
                for kw in range(3):
                    k = kh * 3 + kw
                    cmp_ = T([128, 64], "ds_cmp")
                    V.tensor_copy(cmp_[:].rearrange("p (a b) -> p a b", a=8),
                                  xr3[:, kh:kh + 16:2, kw:kw + 16:2])
                    MM(pds[:, s * 128:(s + 1) * 128],
                       cmp_[:],
                       wt['dsw_pk'][:, k, :],
                       start=(k == 0), stop=(k == 8),
                       skip_group_check=True)
        for s in range(BPC):
            view = pds[:, s * 128:(s + 1) * 128]
            mus = T([64, 1], "ds_mus", F32)
            V.tensor_reduce(mus[:], view, axis=AX.X, op=ALU.add)
            mean = T([64, 1], "ds_mean", F32)
            V.tensor_scalar(mean[:], mus[:], 1.0 / 128, None, op0=ALU.mult)
            sq = T([64, 128], "ds_sq", F32)
            S.activation(sq[:], view, AF.Square)
            ss = T([64, 1], "ds_ss", F32)
            V.tensor_reduce(ss[:], sq[:], axis=AX.X, op=ALU.add)
            m2 = T([64, 1], "ds_m2", F32)
            V.tensor_tensor(m2[:], mean[:], mean[:], op=ALU.mult)
            var = T([64, 1], "ds_var", F32)
            V.scalar_tensor_tensor(var[:], ss[:], 1.0 / 128, m2[:],
                                   op0=ALU.mult, op1=ALU.subtract)
            lv = T([64, 1], "ds_lv", F32)
            S.activation(lv[:], var[:], AF.Ln, bias=epscol[0:64, 0:1])
            rstd = T([64, 1], "ds_rstd", F32)
            S.activation(rstd[:], lv[:], AF.Exp, scale=-0.5)
            xn = T([64, 128], "ds_xn", F32)
            V.tensor_scalar(xn[:], view, mean[:, 0:1], rstd[:, 0:1],
                            op0=ALU.subtract, op1=ALU.mult)
            t1 = T([64, 128], "ds_t1", F32)
            V.tensor_tensor(t1[:], xn[:], wt['ds_ln_wB'][:], op=ALU.mult)
            o1 = T([64, 128], "ds_o1", F32)
            V.tensor_tensor(o1[:], t1[:], wt['ds_ln_bB'][:], op=ALU.add)
            dma(out[s].rearrange("h w c -> (h w) c"), o1[:])

        stk.close()
    return nc, tap_t


# ---------------------------------------------------------------------------
_CACHE = {}


def _get_program(taps=()):
    key = tuple(sorted(taps))
    if key not in _CACHE:
        _CACHE[key] = build_program(taps)
    return _CACHE[key]


def make_inmaps(inputs, taps=()):
    cst = host_constants()
    w = prep_weights(inputs)
    x = np.asarray(inputs['x'], np.float32).reshape(16, C, L).astype(NPBF)
    idx = np.asarray(inputs['sorted_index'], np.int32)
    inv = np.argsort(idx, axis=1, kind='stable').astype(np.int32)
    in_maps = []
    for c in range(NCORES):
        m = {}
        m.update({k: np.ascontiguousarray(v) for k, v in cst.items()})
        m.update({k: np.ascontiguousarray(v) for k, v in w.items()})
        sl = slice(c * BPC, (c + 1) * BPC)
        m['x2'] = np.ascontiguousarray(x[sl])
        m['idx'] = np.ascontiguousarray(idx[sl])
        m['inv'] = np.ascontiguousarray(inv[sl])
        in_maps.append(m)
    return in_maps


def run(inputs, taps=(), trace=False):
    nc, tap_t = _get_program(taps)
    in_maps = make_inmaps(inputs, taps)
    res = run_bass_kernel_spmd(nc, in_maps, list(range(NCORES)), trace=trace)
    outs = np.concatenate([np.asarray(r['out'], np.float32) for r in res.results],
                          axis=0)
    tapd = {}
    for name in taps:
        tapd[name] = [np.asarray(r.get('t_' + name), np.float32)
                      for r in res.results]
    return outs, tapd, res


def kernel(**inputs):
    outs, _, _ = run(inputs)
    return outs

